# revision 2
# baseline (speedup 1.0000x reference)
"""Trainium2 Bass kernel v2 for nn_EntityCell (scatter_memory).

Math (per batch row r, entity e):
    gates = sigmoid(sum_d(s * (h + k)))              [B, E]
    h_tilda = tanh(h @ U + k @ V + (s @ W)[:, None]) [B, E, D]
    updated = h + gates[:, :, None] * h_tilda
    out = updated / sqrt(max(sum_d(updated^2), 1e-12))

Key restructurings vs v1:
  - SWDGE cast-DMA loads (fp32 HBM -> fp16 SBUF) replace ScalarE casts;
    loads for pair n+1 are emitted before pair n's compute so the POOL
    instruction stream never head-of-line-blocks the prefetch.
  - hk = h + k formed once (GpSimd); matmuls become hk@V + h@(U-V) + s@W,
    so k is never transposed and the gate product is one GpSimd op.
  - All d-major transposes on PE (identity stationary), 8-entity PSUM
    tiles, evacuated by ACT (hT) and DVE (hkT/sT): no DMA-xbar traffic,
    DMA stays at the HBM floor.
  - Gates via a row-major GpSimd product + one DVE tensor_reduce
    (no per-entity PE ldweights for the gate reduction).
  - Entity-major matmul groups with a single start/stop per PSUM bank
    (start=True clears the whole bank's has_written bits).
  - 1 Newton rsqrt iteration; fp16 scale output cast-stored to fp32 by
    SWDGE DMA (halves store-side SBUF traffic).
"""

import numpy as np
from contextlib import nullcontext as _nullctx

B, E, D = 8192, 20, 128
N_CORES = 8
B_LOC = B // N_CORES
CHUNK = 128
N_PAIRS = B_LOC // (2 * CHUNK)
EG = 4   # entities per main-matmul PSUM group (bank = 4*128 fp32)
TG = 8   # entities per transpose PSUM tile (bank = 8*128 fp16)

_CACHE = {}


def _build_nc(loop_n=None, ablate=None, cast="swdge", newton_iters=1,
              scl_dve=20, upd_dve=20, hk_eng="gp", psm_bufs=3, psk_bufs=4,
              io_bufs=4, tr_bufs=2, gates_eng="dve", mm_order="entity",
              bn_pair=0, hk_split=1, evac_eng="act", tr_hk="pe", tr_h="pe",
              store16=1):
    import concourse.tile as tile
    from concourse import bacc, mybir
    from concourse.masks import make_identity
    from contextlib import ExitStack

    fp32 = mybir.dt.float32
    fp16 = mybir.dt.float16
    int32 = mybir.dt.int32
    AF = mybir.ActivationFunctionType
    OP = mybir.AluOpType

    nc = bacc.Bacc("TRN2", target_bir_lowering=False, debug=False)
    enc_d = nc.declare_dram_parameter("enc", [B_LOC, D], fp32, isOutput=False)
    prev_d = nc.declare_dram_parameter("prev", [B_LOC, E, D], fp32, isOutput=False)
    keys_d = nc.declare_dram_parameter("keys", [B_LOC, E, D], fp32, isOutput=False)
    u_d = nc.declare_dram_parameter("U", [D, D], fp32, isOutput=False)
    v_d = nc.declare_dram_parameter("V", [D, D], fp32, isOutput=False)
    w_d = nc.declare_dram_parameter("W", [D, D], fp32, isOutput=False)
    out_d = nc.declare_dram_parameter("out", [B_LOC, E, D], fp32, isOutput=True)

    prev_v = prev_d[:].rearrange("(n two p) e d -> n p two (e d)", two=2, p=CHUNK)
    keys_v = keys_d[:].rearrange("(n two p) e d -> n p two (e d)", two=2, p=CHUNK)
    enc_v = enc_d[:].rearrange("(n two p) d -> n p two d", two=2, p=CHUNK)
    out_v = out_d[:].rearrange("(n p) e d -> n p (e d)", p=CHUNK)

    # transpose psum-tile rounds: (start entity, count)
    TR_ROUNDS = [(0, TG), (TG, TG), (2 * TG, E - 2 * TG)]

    with ExitStack() as ctx:
        tc = ctx.enter_context(tile.TileContext(nc))
        const_pool = ctx.enter_context(tc.tile_pool(name="const", bufs=1))
        io_pool = ctx.enter_context(tc.tile_pool(name="io", bufs=io_bufs))
        st_pool = ctx.enter_context(tc.tile_pool(name="st", bufs=2))
        bf_pool = ctx.enter_context(tc.tile_pool(name="bf", bufs=2))
        tr_pool = ctx.enter_context(tc.tile_pool(name="tr", bufs=tr_bufs))
        sm_pool = ctx.enter_context(tc.tile_pool(name="sm", bufs=4))
        psm_pool = ctx.enter_context(tc.tile_pool(name="psm", bufs=psm_bufs, space="PSUM"))
        psg_pool = ctx.enter_context(tc.tile_pool(name="psg", bufs=1, space="PSUM"))
        psk_pool = ctx.enter_context(tc.tile_pool(name="psk", bufs=psk_bufs, space="PSUM"))

        # ---- constants ----
        u32c = const_pool.tile([D, D], fp32)
        v32c = const_pool.tile([D, D], fp32)
        w32c = const_pool.tile([D, D], fp32)
        nc.sync.dma_start(u32c[:], u_d[:])
        nc.sync.dma_start(v32c[:], v_d[:])
        nc.sync.dma_start(w32c[:], w_d[:])
        uv32c = const_pool.tile([D, D], fp32)
        nc.vector.tensor_tensor(uv32c[:], u32c[:], v32c[:], OP.subtract)
        v16c = const_pool.tile([D, D], fp16)
        nc.scalar.copy(v16c[:], v32c[:])
        uv16c = const_pool.tile([D, D], fp16)
        nc.scalar.copy(uv16c[:], uv32c[:])
        w16c = const_pool.tile([D, D], fp16)
        nc.scalar.copy(w16c[:], w32c[:])
        ones16 = const_pool.tile([D, 1], fp16)
        nc.gpsimd.memset(ones16[:], 1.0)
        magic = const_pool.tile([CHUNK, E], int32)
        nc.gpsimd.memset(magic[:], 0x5F3759DF)
        ident16 = const_pool.tile([D, D], fp16)
        make_identity(nc, ident16[:])

        def emit_loads(n):
            if cast == "swdge":
                h16p = io_pool.tile([CHUNK, 2, E, D], fp16, name="h16p")
                nc.gpsimd.dma_start(
                    h16p[:].rearrange("p a e d -> p a (e d)"), prev_v[n]
                )
                k16p = io_pool.tile([CHUNK, 2, E, D], fp16, name="k16p")
                nc.gpsimd.dma_start(
                    k16p[:].rearrange("p a e d -> p a (e d)"), keys_v[n]
                )
                s16p = io_pool.tile([CHUNK, 2, D], fp16, name="s16p")
                nc.gpsimd.dma_start(s16p[:], enc_v[n])
                return h16p, k16p, s16p
            h32p = io_pool.tile([CHUNK, 2, E, D], fp32, name="h32p")
            nc.sync.dma_start(h32p[:].rearrange("p a e d -> p a (e d)"), prev_v[n])
            k32p = io_pool.tile([CHUNK, 2, E, D], fp32, name="k32p")
            nc.sync.dma_start(k32p[:].rearrange("p a e d -> p a (e d)"), keys_v[n])
            s32p = io_pool.tile([CHUNK, 2, D], fp32, name="s32p")
            nc.sync.dma_start(s32p[:], enc_v[n])
            h16p = bf_pool.tile([CHUNK, 2, E, D], fp16, name="h16p")
            nc.scalar.copy(h16p[:], h32p[:])
            k16p = bf_pool.tile([CHUNK, 2, E, D], fp16, name="k16p")
            nc.vector.tensor_copy(k16p[:], k32p[:])
            s16p = bf_pool.tile([CHUNK, 2, D], fp16, name="s16p")
            nc.scalar.copy(s16p[:], s32p[:])
            return h16p, k16p, s16p

        loop_cm = (
            tc.For_i(0, loop_n, 1, hint_engines=tuple(mybir.ALL_ENGINES))
            if loop_n is not None
            else _nullctx()
        )
        cur = emit_loads(0)  # prologue: before the device loop
        with loop_cm:
         for n in range(N_PAIRS):
             # prefetch pair n+1 before any compute of pair n
             if loop_n is not None:
                 nxt = emit_loads((n + 1) % N_PAIRS)
             elif n + 1 < N_PAIRS:
                 nxt = emit_loads(n + 1)
             else:
                 nxt = None
             h16p, k16p, s16p = cur
             cur = nxt

             if ablate == "dma":
                 for half in range(2):
                     nc.sync.dma_start(
                         out=out_v[2 * n + half][:, : E * D // 2],
                         in_=h16p[:, half].rearrange("p e d -> p (e d)").bitcast(fp32),
                     )
                     nc.sync.dma_start(
                         out=out_v[2 * n + half][:, E * D // 2 :],
                         in_=k16p[:, half].rearrange("p e d -> p (e d)").bitcast(fp32),
                     )
                 continue

             for half in range(2):
                 c = 2 * n + half
                 h16 = h16p[:, half]
                 k16 = k16p[:, half]
                 s16 = s16p[:, half]

                 # ---- hk = h + k (row-major fp16) ----
                 hk16 = bf_pool.tile([CHUNK, E, D], fp16, name="hk16")
                 hk_eng_ = nc.gpsimd if hk_eng == "gp" else nc.vector
                 if hk_split:
                     for lo, ne in TR_ROUNDS:
                         hk_eng_.tensor_tensor(
                             hk16[:, lo : lo + ne], h16[:, lo : lo + ne],
                             k16[:, lo : lo + ne], OP.add,
                         )
                 else:
                     hk_eng_.tensor_tensor(hk16[:], h16, k16, OP.add)

                 # ---- transposes (PE w/ psum evac, or DMA xbar) ----
                 hT = tr_pool.tile([D, E, CHUNK], fp16, name="hT")
                 hkT = tr_pool.tile([D, E, CHUNK], fp16, name="hkT")
                 sT = tr_pool.tile([D, CHUNK], fp16, name="sT")
                 ev = nc.scalar.copy if evac_eng == "act" else nc.vector.tensor_copy
                 if tr_h == "xbar":
                     nc.sync.dma_start_transpose(out=hT[:], in_=h16)
                     tp = psk_pool.tile([D, TG, CHUNK], fp16, name="tp")
                     nc.tensor.transpose(tp[:, 0], s16, ident16[:])
                     ev(sT[:], tp[:, 0])
                 else:
                     for ri, (lo, ne) in enumerate(TR_ROUNDS):
                         tp = psk_pool.tile([D, TG, CHUNK], fp16, name="tp")
                         for j in range(ne):
                             nc.tensor.transpose(tp[:, j], h16[:, lo + j], ident16[:])
                         if ri == len(TR_ROUNDS) - 1:
                             nc.tensor.transpose(tp[:, ne], s16, ident16[:])
                             ev(sT[:], tp[:, ne])
                         nc.scalar.copy(hT[:, lo : lo + ne], tp[:, :ne])
                 if tr_hk == "xbar":
                     nc.sync.dma_start_transpose(out=hkT[:], in_=hk16[:])
                 else:
                     for ri, (lo, ne) in enumerate(TR_ROUNDS):
                         tp = psk_pool.tile([D, TG, CHUNK], fp16, name="tp")
                         for j in range(ne):
                             nc.tensor.transpose(tp[:, j], hk16[:, lo + j], ident16[:])
                         ev(hkT[:, lo : lo + ne], tp[:, :ne])

                 if ablate == "xpose":
                     nc.sync.dma_start(
                         out=out_v[c][:, : E * D // 2],
                         in_=hT[:].rearrange("p e d -> p (e d)").bitcast(fp32),
                     )
                     nc.sync.dma_start(
                         out=out_v[c][:, E * D // 2 :],
                         in_=hkT[:].rearrange("p e d -> p (e d)").bitcast(fp32),
                     )
                     continue

                 # ---- gate input t2 = hk * s ----
                 if gates_eng == "dve":
                     # row-major product on GpSimd; reduce over d on DVE
                     t2r = tr_pool.tile([CHUNK, E, D], fp16, name="t2r")
                     s16b = s16.unsqueeze(1).broadcast_to([CHUNK, E, D])
                     nc.gpsimd.tensor_tensor(t2r[:], hk16[:], s16b, OP.mult)
                     g_raw = sm_pool.tile([CHUNK, E], fp32, name="g_raw")
                     nc.vector.tensor_reduce(
                         g_raw[:], t2r[:], mybir.AxisListType.X, OP.add
                     )
                 else:
                     t2T = tr_pool.tile([D, E, CHUNK], fp16, name="t2T")
                     sTb = sT[:].unsqueeze(1).broadcast_to([D, E, CHUNK])
                     nc.gpsimd.tensor_tensor(t2T[:], hkT[:], sTb, OP.mult)

                 # ---- main matmuls ----
                 ht16 = bf_pool.tile([CHUNK, E, D], fp16, name="ht16")
                 g32 = sm_pool.tile([CHUNK, E], fp32, name="g32")
                 if mm_order == "term":
                     pss = []
                     for gi in range(E // EG):
                         ps = psm_pool.tile([CHUNK, EG, D], fp32, name="ps")
                         pss.append(ps)
                         for j in range(EG):
                             e = gi * EG + j
                             # one start=True per bank: start clears the whole
                             # bank's has_written bits
                             nc.tensor.matmul(
                                 ps[:, j], hkT[:, e], v16c[:], start=(j == 0),
                                 stop=False,
                             )
                     for gi in range(E // EG):
                         for j in range(EG):
                             e = gi * EG + j
                             nc.tensor.matmul(
                                 pss[gi][:, j], hT[:, e], uv16c[:], start=False,
                                 stop=False,
                             )
                     if gates_eng == "dve":
                         nc.scalar.activation(g32[:], g_raw[:], AF.Sigmoid)
                     else:
                         gps = psg_pool.tile([CHUNK, E], fp32, name="gps")
                         for e in range(E):
                             nc.tensor.matmul(
                                 gps[:, e : e + 1], t2T[:, e], ones16[:],
                                 start=True, stop=True,
                             )
                         nc.scalar.activation(g32[:], gps[:], AF.Sigmoid)
                     for gi in range(E // EG):
                         for j in range(EG):
                             nc.tensor.matmul(
                                 pss[gi][:, j], sT[:], w16c[:], start=False,
                                 stop=(j == EG - 1),
                             )
                         nc.scalar.activation(
                             ht16[:, gi * EG : (gi + 1) * EG], pss[gi][:], AF.Tanh
                         )
                 else:
                     for gi in range(E // EG):
                         ps = psm_pool.tile([CHUNK, EG, D], fp32, name="ps")
                         for j in range(EG):
                             e = gi * EG + j
                             nc.tensor.matmul(
                                 ps[:, j], hkT[:, e], v16c[:], start=(j == 0),
                                 stop=False,
                             )
                             nc.tensor.matmul(
                                 ps[:, j], hT[:, e], uv16c[:], start=False, stop=False
                             )
                             nc.tensor.matmul(
                                 ps[:, j], sT[:], w16c[:], start=False,
                                 stop=(j == EG - 1),
                             )
                         nc.scalar.activation(
                             ht16[:, gi * EG : (gi + 1) * EG], ps[:], AF.Tanh
                         )
                     if gates_eng == "dve":
                         nc.scalar.activation(g32[:], g_raw[:], AF.Sigmoid)
                     else:
                         gps = psg_pool.tile([CHUNK, E], fp32, name="gps")
                         for e in range(E):
                             nc.tensor.matmul(
                                 gps[:, e : e + 1], t2T[:, e], ones16[:],
                                 start=True, stop=True,
                             )
                         nc.scalar.activation(g32[:], gps[:], AF.Sigmoid)

                 # ---- update u = g*h_tilda + h (in place over ht16) ----
                 u16 = ht16
                 for e in range(E):
                     eng = nc.vector if e < upd_dve else nc.gpsimd
                     eng.scalar_tensor_tensor(
                         u16[:, e], ht16[:, e], g32[:, e : e + 1], h16[:, e],
                         OP.mult, OP.add,
                     )

                 # ---- sum(u^2) via bn_stats ----
                 a32 = sm_pool.tile([CHUNK, E], fp32, name="a32")
                 if bn_pair:
                     # one bn_stats per 2 entities: halves of the 256-elem input
                     # are exactly the two entities -> sumsq_e = 128*mu^2 + M2
                     bn = sm_pool.tile([CHUNK, E // 2, 6], fp32, name="bn")
                     for p in range(E // 2):
                         nc.vector.bn_stats(
                             bn[:, p, :],
                             u16[:, 2 * p : 2 * p + 2].rearrange("p e d -> p (e d)"),
                         )
                     t_a = sm_pool.tile([CHUNK, E], fp32, name="t_a")
                     nc.vector.tensor_tensor(
                         t_a[:].rearrange("p (g t) -> p g t", t=2),
                         bn[:, :, 1:5:3], bn[:, :, 1:5:3], OP.mult,
                     )
                     nc.vector.scalar_tensor_tensor(
                         a32[:].rearrange("p (g t) -> p g t", t=2),
                         t_a[:].rearrange("p (g t) -> p g t", t=2),
                         128.0, bn[:, :, 2:6:3], OP.mult, OP.add,
                     )
                 else:
                     bn = sm_pool.tile([CHUNK, E, 6], fp32, name="bn")
                     for e in range(E):
                         nc.vector.bn_stats(bn[:, e, :], u16[:, e])
                     t_a = sm_pool.tile([CHUNK, E], fp32, name="t_a")
                     nc.vector.tensor_tensor(t_a[:], bn[:, :, 1], bn[:, :, 1], OP.mult)
                     t_b = sm_pool.tile([CHUNK, E], fp32, name="t_b")
                     nc.vector.tensor_tensor(t_b[:], bn[:, :, 4], bn[:, :, 4], OP.mult)
                     t_ab = sm_pool.tile([CHUNK, E], fp32, name="t_ab")
                     nc.vector.tensor_tensor(t_ab[:], t_a[:], t_b[:], OP.add)
                     t_c = sm_pool.tile([CHUNK, E], fp32, name="t_c")
                     nc.vector.tensor_tensor(t_c[:], bn[:, :, 2], bn[:, :, 5], OP.add)
                     nc.vector.scalar_tensor_tensor(
                         a32[:], t_ab[:], 64.0, t_c[:], OP.mult, OP.add
                     )
                 nc.vector.tensor_scalar(a32[:], a32[:], 1e-12, None, op0=OP.max)

                 # ---- r = rsqrt(a): bit-trick seed + Newton ----
                 ti = sm_pool.tile([CHUNK, E], int32, name="ti")
                 nc.vector.tensor_scalar(
                     ti[:], a32[:].bitcast(int32), 1, None,
                     op0=OP.logical_shift_right,
                 )
                 yi = sm_pool.tile([CHUNK, E], int32, name="yi")
                 nc.vector.tensor_tensor(yi[:], magic[:], ti[:], OP.subtract)
                 y = yi[:].bitcast(fp32)
                 for _ in range(newton_iters):
                     y2 = sm_pool.tile([CHUNK, E], fp32, name="y2")
                     nc.vector.tensor_tensor(y2[:], y, y, OP.mult)
                     tt = sm_pool.tile([CHUNK, E], fp32, name="tt")
                     nc.vector.tensor_tensor(tt[:], a32[:], y2[:], OP.mult)
                     ww = sm_pool.tile([CHUNK, E], fp32, name="ww")
                     nc.vector.tensor_scalar(
                         ww[:], tt[:], -0.5, 1.5, op0=OP.mult, op1=OP.add
                     )
                     yn = sm_pool.tile([CHUNK, E], fp32, name="yn")
                     nc.vector.tensor_tensor(yn[:], y, ww[:], OP.mult)
                     y = yn[:]

                 # ---- scale out = u * r and store ----
                 if store16:
                     o16 = st_pool.tile([CHUNK, E, D], fp16, name="o16")
                     for e in range(E):
                         eng = nc.vector if e < scl_dve else nc.gpsimd
                         eng.tensor_scalar(
                             o16[:, e], u16[:, e], y[:, e : e + 1], None, op0=OP.mult
                         )
                     nc.gpsimd.dma_start(
                         out=out_v[c], in_=o16[:].rearrange("p e d -> p (e d)")
                     )
                 else:
                     o32 = st_pool.tile([CHUNK, E, D], fp32, name="o32")
                     for e in range(E):
                         eng = nc.vector if e < scl_dve else nc.gpsimd
                         eng.tensor_scalar(
                             o32[:, e], u16[:, e], y[:, e : e + 1], None, op0=OP.mult
                         )
                     nc.sync.dma_start(
                         out=out_v[c], in_=o32[:].rearrange("p e d -> p (e d)")
                     )

    nc.compile()
    return nc


def _get_nc():
    if "nc" not in _CACHE:
        _CACHE["nc"] = _build_nc()
    return _CACHE["nc"]


def kernel(encoded_sents, prev_states, keys, U, V, W):
    import sys

    if "/opt/trn_rl_repo" not in sys.path:
        sys.path.insert(0, "/opt/trn_rl_repo")
    from concourse.bass_utils import run_bass_kernel_spmd

    nc = _get_nc()
    enc = np.ascontiguousarray(np.asarray(encoded_sents, dtype=np.float32))
    prev = np.ascontiguousarray(np.asarray(prev_states, dtype=np.float32))
    kys = np.ascontiguousarray(np.asarray(keys, dtype=np.float32))
    U = np.ascontiguousarray(np.asarray(U, dtype=np.float32))
    V = np.ascontiguousarray(np.asarray(V, dtype=np.float32))
    W = np.ascontiguousarray(np.asarray(W, dtype=np.float32))

    in_maps = []
    for i in range(N_CORES):
        lo, hi = i * B_LOC, (i + 1) * B_LOC
        in_maps.append(
            {
                "enc": enc[lo:hi],
                "prev": prev[lo:hi],
                "keys": kys[lo:hi],
                "U": U,
                "V": V,
                "W": W,
            }
        )

    res = run_bass_kernel_spmd(nc, in_maps, list(range(N_CORES)))
    out = np.concatenate([res.results[i]["out"] for i in range(N_CORES)], axis=0)
    return out.astype(np.float32)
